# revision 5
# baseline (speedup 1.0000x reference)
"""Trainium2 Bass kernel for nn_CGpool (GNN message passing + CG pooling).

Self-contained: host-side preprocessing (sharding/index prep/layout), one SPMD
Bass program on 8 NeuronCores, host-side unshard of the outputs.

Per core (atoms dst-sharded 8192/core, molecule-aligned):
  h^T [128, 8192] f32 resident in SBUF.
  3x conv layer (transform-then-aggregate):
    M = tanh(h W1 + b1) W2 + b2 per-atom -> DRAM -> AllGather M_full [65536,128].
    Aggregation over in-edges grouped in 256-wide dst groups: per 128-edge
    chunk, dma_gather rows of M_full (lo/hi table views keep indices int16),
    one-hot built on DVE (iota compare), PE matmul accumulates
    S^T = gathered^T @ onehot into PSUM [128,256]; h^T += S^T.
  Head: hl = tanh(h Wc1 + bc1) Wc2 + bc2 (row layout), softmax -> a,
    per-molecule column sums via ones-matmul, a_norm, cg_xyz per molecule.
  cg_adj: edges sharded by mol(e0); a-values fetched via pair table
    ap2 [32768, 128] (pair r = local atoms (r, r+4096) -> int16-safe indices),
    outer products on DVE (4 parity candidates), masked one-hot matmul into
    per-mb0 bin tiles [128 bins, 64].
"""
import sys
sys.path.insert(0, "/opt/trn_rl_repo")
import numpy as np

import concourse.bacc as bacc
import concourse.bass as bass
import concourse.mybir as mybir
import concourse.tile as tile
from concourse import bass_utils

NC = 8
P = 128
D = 128
N_ATOMS = 65536
NPC = N_ATOMS // NC        # 8192
N_CONV = 3
N_CGS = 8
N_MOLS = 128
N_PER = 512
MPC = N_MOLS // NC         # 16
GSZ = 256                  # dst group width
NG = NPC // GSZ            # 32
HALF = N_ATOMS // 2
NIDX = 1024                # dma_gather max batch on this HW

f32 = mybir.dt.float32
i16 = mybir.dt.int16


def _wrap_idx(flat):
    """flat [nops*1024] -> wrapped+replicated [128, nops*64] int16."""
    nops = len(flat) // NIDX
    w = np.concatenate(
        [flat[k * NIDX:(k + 1) * NIDX].reshape(-1, 16).T for k in range(nops)],
        axis=1)
    return np.tile(w, (8, 1)).astype(np.int16)


def prep(atoms_nodes, xyz, bond_edges, edge, mol_id,
         embed, Wu1, bu1, Wu2, bu2, Wc1, bc1, Wc2, bc2):
    atoms_nodes = np.asarray(atoms_nodes, np.int64)
    xyz = np.asarray(xyz, np.float32)
    bond_edges = np.asarray(bond_edges, np.int64)
    edge = np.asarray(edge, np.int64)
    mol_id = np.asarray(mol_id, np.int64)
    embed = np.asarray(embed, np.float32).copy()
    embed[0] = 0.0
    Wu1 = np.asarray(Wu1, np.float32); bu1 = np.asarray(bu1, np.float32)
    Wu2 = np.asarray(Wu2, np.float32); bu2 = np.asarray(bu2, np.float32)
    Wc1 = np.asarray(Wc1, np.float32); bc1 = np.asarray(bc1, np.float32)
    Wc2 = np.asarray(Wc2, np.float32); bc2 = np.asarray(bc2, np.float32)
    assert np.array_equal(mol_id, np.arange(N_ATOMS) // N_PER)

    h0 = embed[atoms_nodes]

    # ---------- message-passing lists (dst-sharded)
    e2 = np.concatenate([bond_edges, bond_edges[:, ::-1]], axis=0)
    dst, src = e2[:, 0], e2[:, 1]
    core_of = dst // NPC
    per_core = []
    for c in range(NC):
        m = core_of == c
        s, dl = src[m], dst[m] - c * NPC
        per_core.append((s, dl // GSZ, dl % GSZ, (s >= HALF).astype(np.int64)))
    nch = np.zeros((NG, 2), np.int64)
    for g in range(NG):
        for hf in range(2):
            mx = max(((pc[1] == g) & (pc[3] == hf)).sum() for pc in per_core)
            nch[g, hf] = (mx + P - 1) // P
    idx_lo, idx_hi, dstloc_cols = [], [], []
    for c in range(NC):
        s, gg, dloc, half = per_core[c]
        lo_p, hi_p, dl_p = [], [], []
        for g in range(NG):
            for hf in range(2):
                m = (gg == g) & (half == hf)
                sv = s[m] - hf * HALF
                L = nch[g, hf] * P
                sp = np.zeros(L, np.int64); sp[:len(sv)] = sv
                dp = np.full(L, 999.0, np.float32); dp[:m.sum()] = dloc[m]
                (lo_p if hf == 0 else hi_p).append(sp)
                dl_p.append(dp.reshape(-1, P))
        idx_lo.append(np.concatenate(lo_p))
        idx_hi.append(np.concatenate(hi_p))
        dstloc_cols.append(np.concatenate(dl_p, axis=0))
    slots_lo, slots_hi = len(idx_lo[0]), len(idx_hi[0])
    nops_lo = (slots_lo + NIDX - 1) // NIDX
    nops_hi = (slots_hi + NIDX - 1) // NIDX
    mp_idx_lo_w = [_wrap_idx(np.concatenate(
        [a, np.zeros(nops_lo * NIDX - len(a), np.int64)])) for a in idx_lo]
    mp_idx_hi_w = [_wrap_idx(np.concatenate(
        [a, np.zeros(nops_hi * NIDX - len(a), np.int64)])) for a in idx_hi]
    mp_dstloc = [a.T.copy().astype(np.float32) for a in dstloc_cols]
    nchunks = dstloc_cols[0].shape[0]

    # chunk schedule: g-major, lo chunks then hi chunks (consumption order)
    sched = []
    cur = [0, 0]
    for g in range(NG):
        n_lo, n_hi = int(nch[g, 0]), int(nch[g, 1])
        k, tot = 0, n_lo + n_hi
        for hf, n in ((0, n_lo), (1, n_hi)):
            for j in range(n):
                sched.append((hf, cur[hf] + j, g, k == 0, k == tot - 1))
                k += 1
            cur[hf] += n

    # ---------- cg_adj lists (sharded by mol(e0) owner == atom owner)
    e0, e1 = edge[:, 0], edge[:, 1]
    mb1 = e1 // N_PER

    def pair_idx(a):       # global pair-row in ap2_full; parity
        return (a // NPC) * (NPC // 2) + (a % NPC) % (NPC // 2), (a % NPC) // (NPC // 2)

    ecore = e0 // NPC
    cg_per_core = []
    for c in range(NC):
        m = ecore == c
        cg_per_core.append((e0[m], e1[m], (e0[m] // N_PER) - c * MPC, mb1[m]))
    cg_nch = np.zeros(MPC, np.int64)
    for t in range(MPC):
        mx = max((pc[2] == t).sum() for pc in cg_per_core)
        cg_nch[t] = (mx + P - 1) // P
    cg_chunks = int(cg_nch.sum())
    cg_nops = (2 * cg_chunks * P + NIDX - 1) // NIDX
    cg_idx_w, cg_binloc_t, cg_masks_t = [], [], []
    for c in range(NC):
        ce0, ce1, cmb0, cmb1 = cg_per_core[c]
        i0_all = np.zeros(cg_chunks * P, np.int64)
        i1_all = np.zeros(cg_chunks * P, np.int64)
        bl_all = np.full(cg_chunks * P, 999.0, np.float32)
        mk_all = np.zeros((cg_chunks * P, 4), np.float32)
        pos = 0
        for t in range(MPC):
            m = cmb0 == t
            a0, a1, b1 = ce0[m], ce1[m], cmb1[m]
            n = len(a0)
            L = int(cg_nch[t]) * P
            r0, x0 = pair_idx(a0)
            r1, x1 = pair_idx(a1)
            i0_all[pos:pos + n] = r0
            i1_all[pos:pos + n] = r1
            bl_all[pos:pos + n] = b1
            mk_all[pos + np.arange(n), x0 * 2 + x1] = 1.0
            pos += L
        inter = np.zeros(cg_nops * NIDX, np.int64)
        for j in range(cg_chunks):
            inter[(2 * j) * P:(2 * j + 1) * P] = i0_all[j * P:(j + 1) * P]
            inter[(2 * j + 1) * P:(2 * j + 2) * P] = i1_all[j * P:(j + 1) * P]
        cg_idx_w.append(_wrap_idx(inter))
        cg_binloc_t.append(bl_all.reshape(-1, P).T.copy())
        cg_masks_t.append(np.transpose(
            mk_all.reshape(-1, P, 4), (1, 0, 2)).reshape(P, -1).copy())

    # ---------- constants
    iota256 = np.tile(np.arange(GSZ, dtype=np.float32), (P, 1))
    bu1T = bu1.T.copy().astype(np.float32)
    bu2rep = np.concatenate([np.tile(bu2[l], (P, 1)) for l in range(N_CONV)], axis=1)
    Wu1s = np.concatenate([Wu1[l] for l in range(N_CONV)], axis=1)
    Wu2s = np.concatenate([Wu2[l] for l in range(N_CONV)], axis=1)

    in_maps = []
    for c in range(NC):
        sl = slice(c * NPC, (c + 1) * NPC)
        xl = xyz[sl]
        xyz_st = np.zeros((P, 64 * 3), np.float32)
        for b in range(64):
            xyz_st[:, 3 * b:3 * b + 3] = xl[b * P:(b + 1) * P]
        in_maps.append({
            "h0T": h0[sl].T.copy().astype(np.float32),
            "mp_idx_lo": mp_idx_lo_w[c],
            "mp_idx_hi": mp_idx_hi_w[c],
            "mp_dstloc": mp_dstloc[c],
            "cg_idx": cg_idx_w[c],
            "cg_binloc": cg_binloc_t[c].astype(np.float32),
            "cg_masks": cg_masks_t[c].astype(np.float32),
            "xyz_st": xyz_st,
            "iota256": iota256,
            "ones_col": np.ones((P, 1), np.float32),
            "ones_row": np.ones((1, P), np.float32),
            "Wu1s": Wu1s, "Wu2s": Wu2s, "bu1T": bu1T, "bu2rep": bu2rep,
            "Wc1": Wc1, "bc1T": bc1[:, None].astype(np.float32),
            "Wc2": Wc2, "bc2rep": np.tile(bc2, (P, 1)).astype(np.float32),
        })
    meta = {"sched": sched, "nops_lo": nops_lo, "nops_hi": nops_hi,
            "nchunks": nchunks, "cg_nch": cg_nch.tolist(),
            "cg_chunks": cg_chunks, "cg_nops": cg_nops}
    return in_maps, meta


def build_nc(meta, reps=1):
    sched = meta["sched"]
    nops_lo, nops_hi = meta["nops_lo"], meta["nops_hi"]
    nchunks = meta["nchunks"]
    cg_nch, cg_chunks, cg_nops = meta["cg_nch"], meta["cg_chunks"], meta["cg_nops"]

    nc = bacc.Bacc("TRN2", target_bir_lowering=False, debug=False)
    ein = lambda n, s, d=f32: nc.dram_tensor(n, s, d, kind="ExternalInput")
    h0T = ein("h0T", [P, NPC])
    mp_idx_lo = ein("mp_idx_lo", [P, nops_lo * NIDX // 16], i16)
    mp_idx_hi = ein("mp_idx_hi", [P, nops_hi * NIDX // 16], i16)
    mp_dstloc = ein("mp_dstloc", [P, nchunks])
    cg_idx = ein("cg_idx", [P, cg_nops * NIDX // 16], i16)
    cg_binloc = ein("cg_binloc", [P, cg_chunks])
    cg_masks = ein("cg_masks", [P, cg_chunks * 4])
    xyz_st = ein("xyz_st", [P, 64 * 3])
    iota256 = ein("iota256", [P, GSZ])
    ones_col = ein("ones_col", [P, 1])
    ones_row = ein("ones_row", [1, P])
    Wu1s = ein("Wu1s", [P, N_CONV * D]); Wu2s = ein("Wu2s", [P, N_CONV * D])
    bu1T = ein("bu1T", [P, N_CONV]); bu2rep = ein("bu2rep", [P, N_CONV * D])
    Wc1 = ein("Wc1", [P, D]); bc1T = ein("bc1T", [P, 1])
    Wc2 = ein("Wc2", [P, N_CGS]); bc2rep = ein("bc2rep", [P, N_CGS])

    hl_out = nc.dram_tensor("hl_out", [NPC, N_CGS], f32, kind="ExternalOutput")
    anorm_out = nc.dram_tensor("anorm_out", [NPC, N_CGS], f32, kind="ExternalOutput")
    cgxyz_out = nc.dram_tensor("cgxyz_out", [P, 3], f32, kind="ExternalOutput")
    cgadj_out = nc.dram_tensor("cgadj_out", [MPC * P, 64], f32, kind="ExternalOutput")

    AF = mybir.ActivationFunctionType
    ALU = mybir.AluOpType

    with tile.TileContext(nc) as tc:
        with tc.tile_pool(name="dram", bufs=1, space="DRAM") as dram, \
             tc.tile_pool(name="cst", bufs=1) as cst, \
             tc.tile_pool(name="hTp", bufs=1) as hTp, \
             tc.tile_pool(name="tTp", bufs=1) as tTp, \
             tc.tile_pool(name="work", bufs=4) as work, \
             tc.tile_pool(name="gt", bufs=6) as gtp, \
             tc.tile_pool(name="oh", bufs=6) as ohp, \
             tc.tile_pool(name="psA", bufs=2, space="PSUM") as psA, \
             tc.tile_pool(name="psG", bufs=3, space="PSUM") as psG, \
             tc.tile_pool(name="psS", bufs=2, space="PSUM") as psS:

            M_loc = dram.tile([NPC, D], f32)
            M_full = dram.tile([N_ATOMS, D], f32)
            ap2_loc = dram.tile([NPC // 2, P], f32)
            ap2_full = dram.tile([N_ATOMS // 2, P], f32)

            def ld(src, shape, tag, dt=f32):
                t = cst.tile(shape, dt, tag=tag)
                nc.sync.dma_start(t[:], src[:])
                return t

            c_iota = ld(iota256, [P, GSZ], "c_iota")
            c_W1 = ld(Wu1s, [P, N_CONV * D], "c_W1")
            c_W2 = ld(Wu2s, [P, N_CONV * D], "c_W2")
            c_bu1 = ld(bu1T, [P, N_CONV], "c_bu1")
            c_bu2 = ld(bu2rep, [P, N_CONV * D], "c_bu2")
            c_Wc1 = ld(Wc1, [P, D], "c_Wc1"); c_bc1 = ld(bc1T, [P, 1], "c_bc1")
            c_Wc2 = ld(Wc2, [P, N_CGS], "c_Wc2")
            c_bc2 = ld(bc2rep, [P, N_CGS], "c_bc2")
            c_onc = ld(ones_col, [P, 1], "c_onc")
            c_onr = ld(ones_row, [1, P], "c_onr")
            c_xyz = ld(xyz_st, [P, 64 * 3], "c_xyz")
            c_dstloc = ld(mp_dstloc, [P, nchunks], "c_dstloc")
            c_ilo = ld(mp_idx_lo, [P, nops_lo * NIDX // 16], "c_ilo", i16)
            c_ihi = ld(mp_idx_hi, [P, nops_hi * NIDX // 16], "c_ihi", i16)
            c_binloc = ld(cg_binloc, [P, cg_chunks], "c_binloc")
            c_masks = ld(cg_masks, [P, cg_chunks * 4], "c_masks")
            c_cgi = ld(cg_idx, [P, cg_nops * NIDX // 16], "c_cgi", i16)

            hT = hTp.tile([P, NPC], f32)
            nc.sync.dma_start(hT[:], h0T[:])
            tT = tTp.tile([P, NPC], f32)

            for _rep in range(reps):
                for l in range(N_CONV):
                    lsl = slice(l * D, (l + 1) * D)
                    # MLP A
                    for ch in range(NPC // 512):
                        csl = slice(ch * 512, (ch + 1) * 512)
                        ps = psA.tile([P, 512], f32, tag="mm")
                        nc.tensor.matmul(ps[:], lhsT=c_W1[:, lsl], rhs=hT[:, csl],
                                         start=True, stop=True)
                        nc.scalar.activation(tT[:, csl], ps[:], AF.Tanh,
                                             bias=c_bu1[:, l:l + 1])
                    # MLP B
                    for b in range(NPC // P):
                        bsl = slice(b * P, (b + 1) * P)
                        ps2 = psA.tile([P, D], f32, tag="mm")
                        nc.tensor.matmul(ps2[:], lhsT=tT[:, bsl], rhs=c_W2[:, lsl],
                                         start=True, stop=True)
                        msb = work.tile([P, D], f32, tag="msb")
                        nc.vector.tensor_tensor(out=msb[:], in0=ps2[:],
                                                in1=c_bu2[:, lsl], op=ALU.add)
                        nc.sync.dma_start(M_loc[bsl, :], msb[:])
                    nc.gpsimd.collective_compute(
                        "AllGather", ALU.bypass,
                        replica_groups=[list(range(NC))],
                        ins=[M_loc.opt()], outs=[M_full.opt()])
                    # aggregation
                    gtiles = {}

                    def ensure_op(hf, op, gtiles=gtiles):
                        if (hf, op) in gtiles:
                            return
                        g = gtp.tile([P, (NIDX // P) * D], f32, tag="g")
                        src = M_full[0:HALF, :] if hf == 0 else M_full[HALF:, :]
                        it = c_ilo if hf == 0 else c_ihi
                        nc.gpsimd.dma_gather(
                            out_ap=g[:].rearrange("p (c e) -> p c e", e=D),
                            in_ap=src,
                            idxs_ap=it[:16, op * (NIDX // 16):(op + 1) * (NIDX // 16)],
                            num_idxs=NIDX, num_idxs_reg=NIDX, elem_size=D)
                        gtiles[(hf, op)] = g

                    psg = None
                    for ci, (hf, scj, g_, first, last) in enumerate(sched):
                        op, col = scj // 8, scj % 8
                        ensure_op(hf, op)
                        gt = gtiles[(hf, op)]
                        o_sb = ohp.tile([P, GSZ], f32, tag="oh")
                        nc.vector.tensor_scalar(
                            out=o_sb[:], in0=c_iota[:],
                            scalar1=c_dstloc[:, ci:ci + 1], scalar2=None,
                            op0=ALU.is_equal)
                        if first:
                            psg = psG.tile([P, GSZ], f32, tag="psg")
                        nc.tensor.matmul(psg[:], lhsT=gt[:, col * D:(col + 1) * D],
                                         rhs=o_sb[:], start=first, stop=last)
                        if last:
                            gsl = slice(g_ * GSZ, (g_ + 1) * GSZ)
                            nc.vector.tensor_tensor(out=hT[:, gsl], in0=hT[:, gsl],
                                                    in1=psg[:], op=ALU.add)

                # ---------------- head ----------------
                for ch in range(NPC // 512):
                    csl = slice(ch * 512, (ch + 1) * 512)
                    ps = psA.tile([P, 512], f32, tag="mm")
                    nc.tensor.matmul(ps[:], lhsT=c_Wc1[:], rhs=hT[:, csl],
                                     start=True, stop=True)
                    nc.scalar.activation(tT[:, csl], ps[:], AF.Tanh,
                                         bias=c_bc1[:, 0:1])
                a_all = hTp.tile([P, 64 * N_CGS], f32, tag="a_all")
                for b in range(64):
                    bsl = slice(b * P, (b + 1) * P)
                    ps3 = psA.tile([P, N_CGS], f32, tag="mm")
                    nc.tensor.matmul(ps3[:], lhsT=tT[:, bsl], rhs=c_Wc2[:],
                                     start=True, stop=True)
                    hl_sb = work.tile([P, N_CGS], f32, tag="hl")
                    nc.vector.tensor_tensor(out=hl_sb[:], in0=ps3[:],
                                            in1=c_bc2[:], op=ALU.add)
                    nc.sync.dma_start(hl_out[bsl, :], hl_sb[:])
                    esb = work.tile([P, N_CGS], f32, tag="esb")
                    ssum = work.tile([P, 1], f32, tag="ssum")
                    nc.scalar.activation(esb[:], hl_sb[:], AF.Exp,
                                         accum_out=ssum[:, 0:1])
                    rs = work.tile([P, 1], f32, tag="rs")
                    nc.vector.reciprocal(rs[:], ssum[:])
                    nc.vector.tensor_scalar(
                        out=a_all[:, b * N_CGS:(b + 1) * N_CGS], in0=esb[:],
                        scalar1=rs[:, 0:1], scalar2=None, op0=ALU.mult)
                # colsum per molecule -> reciprocal -> broadcast
                ps_cs = psS.tile([1, P], f32, tag="ss")
                for mm in range(MPC):
                    for i in range(4):
                        b = mm * 4 + i
                        nc.tensor.matmul(
                            ps_cs[0:1, mm * N_CGS:(mm + 1) * N_CGS],
                            lhsT=c_onc[:],
                            rhs=a_all[:, b * N_CGS:(b + 1) * N_CGS],
                            start=(i == 0), stop=(i == 3))
                cs_sb = work.tile([1, P], f32, tag="cs")
                nc.vector.tensor_copy(cs_sb[:], ps_cs[0:1, :])
                rcs = work.tile([1, P], f32, tag="rcs")
                nc.vector.reciprocal(rcs[:], cs_sb[:])
                ps_b = psS.tile([P, P], f32, tag="ss")
                nc.tensor.matmul(ps_b[:], lhsT=c_onr[:], rhs=rcs[:],
                                 start=True, stop=True)
                rb = work.tile([P, P], f32, tag="rb")
                nc.vector.tensor_copy(rb[:], ps_b[:])
                an_all = tTp.tile([P, 64 * N_CGS], f32, tag="an_all")
                for b in range(64):
                    mm = b // 4
                    nc.vector.tensor_tensor(
                        out=an_all[:, b * N_CGS:(b + 1) * N_CGS],
                        in0=a_all[:, b * N_CGS:(b + 1) * N_CGS],
                        in1=rb[:, mm * N_CGS:(mm + 1) * N_CGS], op=ALU.mult)
                nc.sync.dma_start(
                    anorm_out[:].rearrange("(b p) j -> p b j", p=P),
                    an_all[:].rearrange("p (b j) -> p b j", j=N_CGS))
                # cg_xyz
                cgx = work.tile([N_CGS, MPC * 3], f32, tag="cgx")
                for mm in range(MPC):
                    psx = psS.tile([N_CGS, 3], f32, tag="ss")
                    for i in range(4):
                        b = mm * 4 + i
                        nc.tensor.matmul(psx[:],
                                         lhsT=an_all[:, b * N_CGS:(b + 1) * N_CGS],
                                         rhs=c_xyz[:, b * 3:(b + 1) * 3],
                                         start=(i == 0), stop=(i == 3))
                    nc.vector.tensor_copy(cgx[:, mm * 3:(mm + 1) * 3], psx[:])
                nc.sync.dma_start(
                    cgxyz_out[:].rearrange("(m i) k -> i m k", i=N_CGS),
                    cgx[:].rearrange("i (m k) -> i m k", k=3))

                # ---------------- cg_adj ----------------
                # pair table ap2: row r (local) = [a[r] pad56 | a[r+4096] pad56]
                zt = work.tile([P, 1024], f32, tag="zt")
                nc.gpsimd.memset(zt[:], 0.0)
                flat = ap2_loc[:].rearrange("r x -> (r x)")
                for k in range(4):
                    nc.sync.dma_start(
                        flat[k * P * 1024:(k + 1) * P * 1024]
                        .rearrange("(p x) -> p x", p=P), zt[:])
                # one affine DMA: a_all[p, (x bb j)] -> ap2 rows bb*128+p, col x*64+j
                dst = ap2_loc[:].rearrange("(bb p) (x j) -> p x bb j",
                                           p=P, x=2)[:, :, :, 0:N_CGS]
                src = a_all[:].rearrange("p (x bb j) -> p x bb j", x=2, j=N_CGS)
                nc.sync.dma_start(dst, src)
                nc.gpsimd.collective_compute(
                    "AllGather", ALU.bypass,
                    replica_groups=[list(range(NC))],
                    ins=[ap2_loc.opt()], outs=[ap2_full.opt()])
                cg_gt = {}

                def ensure_cg(op, cg_gt=cg_gt):
                    if op in cg_gt:
                        return
                    g = gtp.tile([P, (NIDX // P) * P], f32, tag="g")
                    nc.gpsimd.dma_gather(
                        out_ap=g[:].rearrange("p (c e) -> p c e", e=P),
                        in_ap=ap2_full[:],
                        idxs_ap=c_cgi[:16, op * (NIDX // 16):(op + 1) * (NIDX // 16)],
                        num_idxs=NIDX, num_idxs_reg=NIDX, elem_size=P)
                    cg_gt[op] = g

                ci = 0
                for t in range(MPC):
                    psbin = psG.tile([P, 64], f32, tag="psg")
                    nct = int(cg_nch[t])
                    for j in range(nct):
                        s0, s1 = 2 * ci, 2 * ci + 1
                        ensure_cg(s0 // 8)
                        ensure_cg(s1 // 8)
                        g0, g1 = cg_gt[s0 // 8], cg_gt[s1 // 8]
                        b0, b1 = (s0 % 8) * P, (s1 % 8) * P
                        p4 = ohp.tile([P, 4 * 64], f32, tag="p4")
                        for x in range(2):
                            for y in range(2):
                                cnd = x * 2 + y
                                a0ap = g0[:, b0 + x * 64:b0 + x * 64 + N_CGS, None] \
                                    .to_broadcast([P, N_CGS, N_CGS])
                                a1ap = g1[:, None, b1 + y * 64:b1 + y * 64 + N_CGS] \
                                    .to_broadcast([P, N_CGS, N_CGS])
                                nc.vector.tensor_tensor(
                                    out=p4[:, cnd * 64:(cnd + 1) * 64]
                                    .rearrange("p (i j) -> p i j", i=N_CGS),
                                    in0=a0ap, in1=a1ap, op=ALU.mult)
                        for cnd in range(4):
                            o_c = ohp.tile([P, P], f32, tag="ohc")
                            nc.vector.tensor_scalar(
                                out=o_c[:], in0=c_iota[:, 0:P],
                                scalar1=c_binloc[:, ci:ci + 1],
                                scalar2=c_masks[:, ci * 4 + cnd:ci * 4 + cnd + 1],
                                op0=ALU.is_equal, op1=ALU.mult)
                            nc.tensor.matmul(
                                psbin[:], lhsT=o_c[:],
                                rhs=p4[:, cnd * 64:(cnd + 1) * 64],
                                start=(j == 0 and cnd == 0),
                                stop=(j == nct - 1 and cnd == 3))
                        ci += 1
                    ob = work.tile([P, 64], f32, tag="obin")
                    nc.vector.tensor_copy(ob[:], psbin[:])
                    nc.sync.dma_start(cgadj_out[t * P:(t + 1) * P, :], ob[:])
    nc.compile()
    return nc


def kernel(**inputs):
    in_maps, meta = prep(**inputs)
    ncobj = build_nc(meta)
    res = bass_utils.run_bass_kernel_spmd(
        ncobj, in_maps, core_ids=list(range(NC)))
    mol_id = np.asarray(inputs["mol_id"], np.int64)
    hl = np.concatenate([r["hl_out"] for r in res.results], axis=0)
    anorm_vals = np.concatenate([r["anorm_out"] for r in res.results], axis=0)
    cg_xyz = np.concatenate([r["cgxyz_out"] for r in res.results], axis=0)
    a_norm = np.zeros((N_ATOMS, N_MOLS * N_CGS), np.float32)
    cols = mol_id[:, None] * N_CGS + np.arange(N_CGS)[None, :]
    a_norm[np.arange(N_ATOMS)[:, None], cols] = anorm_vals
    cga = np.concatenate([r["cgadj_out"] for r in res.results], axis=0)
    cga = cga.reshape(N_MOLS, N_MOLS, N_CGS, N_CGS)
    cg_adj = np.transpose(cga, (0, 2, 1, 3)).reshape(
        N_MOLS * N_CGS, N_MOLS * N_CGS).astype(np.float32)
    return (hl, cg_xyz, a_norm, cg_adj)
